# revision 6
# baseline (speedup 1.0000x reference)
"""BitLinear (ternary weight quant + per-token int8 activation quant + GEMM)
Trainium2 Bass/Tile kernel, 8-core SPMD.

Sharding: tokens (B*S = 8192) split 8 ways; weight replicated per core.
Each core quantizes a distinct 512-row slice of W (plus a tiny AllReduce
for the global mean(|W|) scale), then an AllGather shares the quantized,
transposed weight.

v2 layout: the PE does ONLY the 2048 N=512 bf16 GEMM matmuls per body
(~236ns/MM is the fused-LDW floor measured on this HW). All transposes
go through the xbar DMA-transpose (one batched [128,4096] instruction
per tile, out[q,ib,t] = in[t, ib*128+q], HW-verified). PSUM: all 8
banks rotate as GEMM accumulators. Activation quantization is software-
pipelined in two half-sized xqT ping-pong buffers so it hides under the
opposite half's GEMM across loop iterations.

Math notes (exactness):
  - a_q in [-127,127] and w_q in {-1,0,1} are exact in bf16; the PE
    accumulates fp32 integer partial sums < 2^24, so the GEMM is exact.
  - round-to-nearest-even via the fp32 magic-number trick (+1.5*2^23).
  - clip(round(w/s),-1,1) == sign(round(w/s)) because |w/s| <= 2, so the
    ACT Sign function performs unshift+clip+cast in one op.
"""

import numpy as np

B, S, D = 2, 4096, 4096
NCORES = 8
T = B * S                  # 8192 tokens
TSH = T // NCORES          # 1024 tokens per core
WSL = D // NCORES          # 512 weight rows per core for quant + mean partial
P = 128
MAGIC = 1.5 * 2**23        # 12582912.0; forces RNE-to-integer in fp32
EPS = 1e-8
QMAX = 127.0
NELEM = float(D * D)

NT = TSH // P              # 8 token tiles
NI = D // P                # 32 contraction blocks
NS = NCORES                # 8 output slices
OSL = D // NS              # 512 output cols per slice
NC_W = WSL // P            # 4 weight row-chunks per core slice
NH = 2                     # xqT halves (ping-pong)
NTH = NT // NH             # 4 token tiles per half

_CACHE: dict = {}


def _build(reps=1, variant="v2h"):
    import concourse.mybir as mybir
    import concourse.tile as tile
    from concourse import bacc

    f32 = mybir.dt.float32
    bf16 = mybir.dt.bfloat16
    X = mybir.AxisListType.X

    nc = bacc.Bacc(
        "TRN2", target_bir_lowering=False, debug=False, num_devices=NCORES
    )

    xs = nc.dram_tensor("xs", [TSH, D], f32, kind="ExternalInput").ap()
    wslice = nc.dram_tensor("wslice", [WSL, D], f32, kind="ExternalInput").ap()
    y = nc.dram_tensor("y", [TSH, D], f32, kind="ExternalOutput").ap()

    with tile.TileContext(nc) as tc:
        with (
            tc.tile_pool(name="stage", bufs=2) as stage,
            tc.tile_pool(name="xb", bufs=2) as xb_pool,
            tc.tile_pool(name="xqt", bufs=2) as xqt_pool,
            tc.tile_pool(name="wqt", bufs=2) as wqt_pool,
            tc.tile_pool(name="comb", bufs=2) as comb_pool,
            tc.tile_pool(name="ysb", bufs=4) as ysb_pool,
            tc.tile_pool(name="small", bufs=1) as small,
            tc.tile_pool(name="py", bufs=8, space="PSUM") as psum_y,
            tc.tile_pool(name="dram", bufs=1, space="DRAM") as dram,
        ):
            negm = small.tile([P, 1], f32, tag="negm")
            nc.vector.memset(negm, -MAGIC)

            # ---- Phase A: partial sum of |wslice|, AllReduce -> w_scale ----
            partials = small.tile([P, NC_W], f32, tag="partials")
            for c in range(NC_W):
                st = stage.tile([P, D], f32, tag="stage")
                nc.sync.dma_start(st, wslice[c * P:(c + 1) * P, :])
                t8 = small.tile([P, 8], f32, tag="t8")
                nc.vector.tensor_reduce(
                    t8, st.rearrange("p (a b) -> p a b", b=512), axis=X,
                    op=mybir.AluOpType.add, apply_absolute_value=True,
                )
                nc.vector.reduce_sum(partials[:, c:c + 1], t8, axis=X)
            pcol = small.tile([P, 1], f32, tag="pcol")
            nc.vector.reduce_sum(pcol, partials, axis=X)

            bounce_in = dram.tile([P, 1], f32, tag="cc_in")
            bounce_out = dram.tile([P, 1], f32, tag="cc_out")
            nc.sync.dma_start(bounce_in, pcol)
            nc.gpsimd.collective_compute(
                "AllReduce",
                mybir.AluOpType.add,
                replica_groups=[list(range(NCORES))],
                ins=[bounce_in.opt()],
                outs=[bounce_out.opt()],
            )
            srow = small.tile([1, P], f32, tag="srow")
            nc.sync.dma_start(srow, bounce_out.rearrange("p one -> one p"))
            stot = small.tile([1, 1], f32, tag="stot")
            nc.vector.reduce_sum(stot, srow, axis=X)
            # w_scale = mean + EPS ; also 1/w_scale and w_scale/127
            ws = small.tile([1, 1], f32, tag="ws")
            nc.vector.tensor_scalar(
                ws, stot, 1.0 / NELEM, EPS,
                op0=mybir.AluOpType.mult, op1=mybir.AluOpType.add,
            )
            wr = small.tile([1, 1], f32, tag="wr")
            nc.vector.reciprocal(wr, ws)
            w127 = small.tile([1, 1], f32, tag="w127")
            nc.vector.tensor_scalar_mul(w127, ws, 1.0 / QMAX)
            wr_col = small.tile([P, 1], f32, tag="wr_col")
            nc.gpsimd.partition_broadcast(wr_col, wr)
            w127_col = small.tile([P, 1], f32, tag="w127_col")
            nc.gpsimd.partition_broadcast(w127_col, w127)

            # ---- Phase A2: quantize + transpose own W slice, AllGather ----
            # wqo[q, ib, o] = sign(round(w[o_global, ib*128+q] / ws))
            wqo = wqt_pool.tile([P, NI, OSL], bf16, tag="wqt")
            for c in range(NC_W):
                st = stage.tile([P, D], f32, tag="stage")
                nc.sync.dma_start(st, wslice[c * P:(c + 1) * P, :])
                # st <- st * (1/w_scale) + MAGIC (RNE to integer + shift)
                nc.vector.tensor_scalar(
                    st, st, wr_col, MAGIC,
                    op0=mybir.AluOpType.mult, op1=mybir.AluOpType.add,
                )
                sgn = xb_pool.tile([P, D], bf16, tag="xb")
                # sign(v - MAGIC) == clip(round(w/ws), -1, 1); bf16 out
                nc.scalar.activation(
                    sgn, st, mybir.ActivationFunctionType.Sign, bias=negm,
                )
                nc.scalar.dma_start(
                    wqo[:, :, c * P:(c + 1) * P], sgn, transpose=True,
                )
            ag_in = dram.tile([NI, P, OSL], bf16, tag="ag_in")
            nc.sync.dma_start(ag_in.rearrange("b p o -> p b o"), wqo)
            ag_out = dram.tile(
                [NCORES, NI, P, OSL], bf16, tag="ag_out", addr_space="Shared",
            )
            nc.gpsimd.collective_compute(
                "AllGather",
                mybir.AluOpType.bypass,
                replica_groups=[list(range(NCORES))],
                ins=[ag_in.opt()],
                outs=[ag_out.opt()],
            )
            # Copy the gathered weights to LOCAL DRAM once: body reads from
            # Shared space concurrent with xbar transposes wedge the device
            # (HW-bisected); local-DRAM reads with the same pattern are fine.
            wql = dram.tile([NCORES, NI, P, OSL], bf16, tag="wql")
            for s in range(NS):
                nc.sync.dma_start(wql[s], ag_out[s])

            # ---- Body: per-iteration phases B (x quant) + C/D (GEMM) ----
            def quant_half(h, xqt_t, comb_t):
                """Quantize token tiles h*NTH..h*NTH+NTH-1 into xqt_t."""
                for tl in range(NTH):
                    t = h * NTH + tl
                    st = stage.tile([P, D], f32, tag="stage")
                    nc.sync.dma_start(st, xs[t * P:(t + 1) * P, :])
                    amax = small.tile([P, 1], f32, tag="amax")
                    nc.vector.tensor_reduce(
                        amax, st, axis=X, op=mybir.AluOpType.max,
                        apply_absolute_value=True,
                    )
                    a_scale = small.tile([P, 1], f32, tag="a_scale")
                    nc.vector.tensor_scalar_add(a_scale, amax, EPS)
                    arec = small.tile([P, 1], f32, tag="arec")
                    nc.vector.reciprocal(arec, a_scale)
                    r127 = small.tile([P, 1], f32, tag="r127")
                    nc.vector.tensor_scalar_mul(r127, arec, QMAX)
                    # comb = a_scale * w_scale / 127 (per token)
                    nc.vector.tensor_scalar(
                        comb_t[:, tl:tl + 1], a_scale, w127_col, None,
                        op0=mybir.AluOpType.mult,
                    )
                    # st <- st * r127 + MAGIC  (RNE to integer + shift)
                    nc.vector.tensor_scalar(
                        st, st, r127, MAGIC,
                        op0=mybir.AluOpType.mult, op1=mybir.AluOpType.add,
                    )
                    # unshift, cast to bf16
                    xbt = xb_pool.tile([P, D], bf16, tag="xb")
                    nc.scalar.activation(
                        xbt, st, mybir.ActivationFunctionType.Identity,
                        bias=negm,
                    )
                    # batched xbar transpose: xqt[q, ib, tl*P+j] = xbt[j, ib*128+q]
                    nc.scalar.dma_start(
                        xqt_t[:, :, tl * P:(tl + 1) * P], xbt, transpose=True,
                    )

            def gemm_half(h, xqt_t, comb_t):
                for s in range(NS):
                    wqt = wqt_pool.tile([P, NI, OSL], bf16, tag="wqt")
                    if variant == "dbg_nw":
                        nc.vector.memset(wqt, 1.0)
                    elif variant == "dbg_sh":
                        # timing-only: read local (non-Shared) DRAM instead
                        nc.sync.dma_start(
                            wqt, ag_in.rearrange("b p o -> p b o")
                        )
                    else:
                        nc.sync.dma_start(
                            wqt, wql[s].rearrange("b p o -> p b o")
                        )
                    for tl in range(NTH):
                        t = h * NTH + tl
                        py = psum_y.tile([P, OSL], f32, tag="py")
                        for i in range(NI):
                            nc.tensor.matmul(
                                py,
                                lhsT=xqt_t[:, i, tl * P:(tl + 1) * P],
                                rhs=wqt[:, i, :],
                                start=(i == 0),
                                stop=(i == NI - 1),
                            )
                        yt = ysb_pool.tile([P, OSL], f32, tag="ysb")
                        nc.scalar.mul(yt, py, comb_t[:, tl:tl + 1])
                        nc.sync.dma_start(
                            y[t * P:(t + 1) * P, s * OSL:(s + 1) * OSL], yt
                        )

            def gemm_s_outer(xqt_a, comb_a, xqt_b, comb_b):
                for s in range(NS):
                    wqt = wqt_pool.tile([P, NI, OSL], bf16, tag="wqt")
                    nc.sync.dma_start(wqt, wql[s].rearrange("b p o -> p b o"))
                    for t in range(NT):
                        h, tl = divmod(t, NTH)
                        xqt_t = xqt_a if h == 0 else xqt_b
                        comb_t = comb_a if h == 0 else comb_b
                        py = psum_y.tile([P, OSL], f32, tag="py")
                        for i in range(NI):
                            nc.tensor.matmul(
                                py,
                                lhsT=xqt_t[:, i, tl * P:(tl + 1) * P],
                                rhs=wqt[:, i, :],
                                start=(i == 0),
                                stop=(i == NI - 1),
                            )
                        yt = ysb_pool.tile([P, OSL], f32, tag="ysb")
                        nc.scalar.mul(yt, py, comb_t[:, tl:tl + 1])
                        nc.sync.dma_start(
                            y[t * P:(t + 1) * P, s * OSL:(s + 1) * OSL], yt
                        )

            def body():
                if variant == "dbg_a2":
                    return
                xqt_a = xqt_pool.tile([P, NI, NTH * P], bf16, tag="xqt")
                comb_a = comb_pool.tile([P, NTH], f32, tag="comb")
                xqt_b = xqt_pool.tile([P, NI, NTH * P], bf16, tag="xqt")
                comb_b = comb_pool.tile([P, NTH], f32, tag="comb")
                if variant == "dbg_g":
                    nc.vector.memset(xqt_a, 1.0)
                    nc.vector.memset(comb_a, 1.0)
                    nc.vector.memset(xqt_b, 1.0)
                    nc.vector.memset(comb_b, 1.0)
                else:
                    quant_half(0, xqt_a, comb_a)
                    quant_half(1, xqt_b, comb_b)
                if variant in ("dbg_q",):
                    return
                if variant == "v2s":
                    gemm_s_outer(xqt_a, comb_a, xqt_b, comb_b)
                else:
                    gemm_half(0, xqt_a, comb_a)
                    gemm_half(1, xqt_b, comb_b)

            if reps == 1:
                body()
            else:
                with tc.For_i(0, reps, 1):
                    body()

    nc.compile()
    return nc


def _get_nc(reps=1, variant="v2h"):
    key = f"nc{reps}-{variant}"
    if key not in _CACHE:
        _CACHE[key] = _build(reps, variant)
    return _CACHE[key]


def make_in_maps(x, weight):
    x = np.ascontiguousarray(np.asarray(x, dtype=np.float32))
    weight = np.ascontiguousarray(np.asarray(weight, dtype=np.float32))
    xf = x.reshape(T, D)
    return [
        {
            "xs": xf[c * TSH:(c + 1) * TSH],
            "wslice": weight[c * WSL:(c + 1) * WSL],
        }
        for c in range(NCORES)
    ]


def run(x, weight, trace=False, variant="v2h", reps=1):
    from concourse.bass_utils import run_bass_kernel_spmd

    nc = _get_nc(reps, variant)
    in_maps = make_in_maps(x, weight)
    res = run_bass_kernel_spmd(
        nc, in_maps, core_ids=list(range(NCORES)), trace=trace
    )
    yf = np.concatenate([res.results[c]["y"] for c in range(NCORES)], axis=0)
    return yf.reshape(B, S, D), res


def kernel(x, weight):
    out, _ = run(x, weight, trace=False)
    return out
